# revision 32
# baseline (speedup 1.0000x reference)
"""Trainium2 Bass kernel for nn_ControlPolicy (T=4096, B=256, N=64, K=2, A=16).

Sharding: data-parallel over batch B across 8 NeuronCores (32 rows/core);
tiny parameters replicated.

v2 design notes (per core):
  LayerNorm is never applied to the full [T,B,N] tensor.  Raw x is
  transposed (PE) and projected (PE matmuls); per-(t,b) mean and E[x^2]
  ride along as extra rows of the smalls matmul chain (ones/64 lhsT
  columns).  Normalization is applied to the 16x smaller projected
  features: e_norm = (W@x - mu*(W@1)) * inv, with the mu-term folded in as
  one more matmul on the accumulation chain and inv broadcast across
  (b,a)-rows by a PE matmul.

  K is collapsed before the scans: the projection emits D = e0-e1 and
  S = e1 feature sets (lhsT rows differenced on the host); the PID chain
  (alpha-filter scan, integrator scan, kp/ki/kd combination) is linear, so
  C = ykS + w0 * ykD.

  The only nonlinear recurrence (a, D-state) runs as an overlap-save
  chunked sweep (R=64, W=24 warm-up, validated ~7e-4 rel): all 64 chunks
  advance together, split into even/odd groups for two independent
  dependency chains.  The anti-windup D-state is small (|kappa*D|<=0.01)
  and slow (lam2=0.88), so it is only updated once per DP=8-step block
  with geometric-sum-corrected coefficients (validated: no error change),
  and tanh(C - D) is batched: one TT + one Activation cover a whole block
  for all chunks, precomputed a block ahead.  The per-step loop-carried
  chain is only rr -> q -> rr via q(j) = p(j) - rate*rr(j-1) with
  p(j) = h(j) - a(j-2) materialized two steps early on the Pool engine.
"""
import math
import numpy as np
from contextlib import ExitStack

import concourse.bass as bass
import concourse.bacc as bacc
import concourse.tile as tile
from concourse import mybir
from concourse.bass_utils import run_bass_kernel_spmd
from concourse.masks import make_identity

F32 = mybir.dt.float32
F16 = mybir.dt.float16
I32 = mybir.dt.int32
OP = mybir.AluOpType
AF = mybir.ActivationFunctionType
AX = mybir.AxisListType

T_FULL = 4096
B_FULL = 256
N = 64
K = 2
A = 16
NCORES = 8
BL = B_FULL // NCORES          # 32
LN_EPS = 1e-5
TWO_PI = float(np.float32(2.0 * np.pi))

R = 64                          # sweep chunk length
W = 24                          # sweep warm-up
DP = 8                          # sweep D-state update period
NSUP = 4                        # supersets of 8 b-rows


def _sigmoid(x): return 1.0 / (1.0 + math.exp(-x))
def _softplus(x): return math.log1p(math.exp(x))


def _coeffs(inputs):
    f = lambda k: float(np.asarray(inputs[k], np.float64))
    alpha = _sigmoid(f("filter_alpha_logit"))
    leak = _sigmoid(f("int_leak_logit"))
    beta = _sigmoid(f("act_beta_logit"))
    rate = 0.25 * _sigmoid(f("rate_limit_raw"))
    aw = _softplus(f("aw_gain_raw"))
    omega_base = _softplus(f("phase_omega_raw")) + 0.001

    kp_a = np.log1p(np.exp(np.asarray(inputs["kp_raw"], np.float64)))
    ki_a = np.log1p(np.exp(np.asarray(inputs["ki_raw"], np.float64)))
    kd_a = np.log1p(np.exp(np.asarray(inputs["kd_raw"], np.float64)))
    for nm, arr in (("kp", kp_a), ("ki", ki_a), ("kd", kd_a)):
        assert np.allclose(arr, arr.flat[0], rtol=1e-12), f"{nm} not uniform"
    kp, ki, kd = float(kp_a.flat[0]), float(ki_a.flat[0]), float(kd_a.flat[0])

    lam2 = 1.0 - leak
    c1 = kp + kd
    kappa = ki * aw * lam2
    co = dict(
        alpha=alpha, lam=1.0 - alpha, lam2=lam2, beta=beta, rate=rate,
        omega_base=omega_base, c1=c1, kappa=kappa,
        s2=beta / (rate + 1e-6), ki_c1=ki / c1, kd_c1=kd / c1,
        kr=kappa * rate, kb=kappa * beta,
    )

    ln_w = np.asarray(inputs["ln_w"], np.float64)
    ln_b = np.asarray(inputs["ln_b"], np.float64)
    ws = np.asarray(inputs["w_state"], np.float64)
    wp = np.asarray(inputs["w_phase"], np.float64)
    b_err = np.asarray(inputs["b_err"], np.float64)
    gw = np.asarray(inputs["gate_w"], np.float64)
    gb = np.asarray(inputs["gate_b"], np.float64)
    pos = np.asarray(inputs["phase_omega_state"], np.float64)

    # feature row sets: D = k0 - k1, S = k1
    WF = [c1 * alpha * (ws[0] - ws[1]) * ln_w,      # [A, N] each
          c1 * alpha * ws[1] * ln_w]
    wpos = alpha * pos * ln_w                        # [N]
    wgd = alpha * (gw[0] - gw[1]) * ln_w
    wpF = [c1 * (wp[0] - wp[1]), c1 * wp[1]]         # [A, 2]
    beF = [c1 * (b_err[0] - b_err[1]), c1 * b_err[1]]

    # fp16-cast weights; row sums for the mu-correction computed from the
    # cast values so the correction matches the matmul exactly
    WFh = [w.astype(np.float16) for w in WF]
    wposh = wpos.astype(np.float16)
    wgdh = wgd.astype(np.float16)
    p1F = [w.astype(np.float64).sum(axis=1) for w in WFh]   # [A] each
    p1pos = float(wposh.astype(np.float64).sum())
    p1gd = float(wgdh.astype(np.float64).sum())

    # e-projection lhsT [128, 2F*4pw*128]: block (F, pw) maps pair pw's
    # partitions (bw2, n64) onto out rows (b8, a16) = 16*(2*pw+bw) + a
    we = np.zeros((128, 2 * 4 * 128), np.float64)
    for F in range(2):
        for pw in range(4):
            base = (F * 4 + pw) * 128
            for bw in range(2):
                for a in range(A):
                    we[bw * 64:(bw + 1) * 64,
                       base + 16 * (2 * pw + bw) + a] = WFh[F][a, :]
    # mu-correction lhsT [32, 4s*2F*128]: rhs = mu_h (full 32 partitions)
    mue = np.zeros((32, NSUP * 2 * 128), np.float64)
    for s in range(NSUP):
        for F in range(2):
            base = (s * 2 + F) * 128
            for bb in range(8):
                for a in range(A):
                    mue[8 * s + bb, base + 16 * bb + a] = -p1F[F][a]
    # smalls+stats chain lhsT [128, 16p*96]: out rows 0-31 pos, 32-63 gdiff,
    # 64-95 mu (ones/64)
    wsm = np.zeros((128, 16 * 96), np.float64)
    for p in range(16):
        for bw in range(2):
            for n in range(N):
                r0 = bw * 64 + n
                wsm[r0, p * 96 + 2 * p + bw] = wposh[n]
                wsm[r0, p * 96 + 32 + 2 * p + bw] = wgdh[n]
                wsm[r0, p * 96 + 64 + 2 * p + bw] = 1.0 / N
    # sq chain lhsT [128, 16p*32]: out rows 96-127 = E[x^2]
    wsq = np.zeros((128, 16 * 32), np.float64)
    for p in range(16):
        for bw in range(2):
            wsq[bw * 64:(bw + 1) * 64, p * 32 + 2 * p + bw] = 1.0 / N
    # smalls mu-correction lhsT [32, 64]: rhs = mu_h
    musm = np.zeros((32, 64), np.float64)
    for b in range(32):
        musm[b, b] = -p1pos
        musm[b, 32 + b] = -p1gd
    # inv broadcast lhsT [32, 4s*128] (rhs = inv_h) and [32, 64]
    binv = np.zeros((32, NSUP * 128), np.float64)
    for s in range(NSUP):
        for bb in range(8):
            binv[8 * s + bb, s * 128 + 16 * bb:s * 128 + 16 * (bb + 1)] = 1.0
    binv64 = np.zeros((32, 64), np.float64)
    for b in range(32):
        binv64[b, b] = 1.0
        binv64[b, 32 + b] = 1.0
    # phase-feature lhsT [64, 4s*2F*128]: rhs = sc (sin rows 0-31, cos 32-63)
    eph = np.zeros((64, NSUP * 2 * 128), np.float64)
    for s in range(NSUP):
        for F in range(2):
            base = (s * 2 + F) * 128
            for bb in range(8):
                for a in range(A):
                    eph[8 * s + bb, base + 16 * bb + a] = wpF[F][a, 0]
                    eph[32 + 8 * s + bb, base + 16 * bb + a] = wpF[F][a, 1]
    # w0 broadcast lhsT [32, 4s*128]
    w0b = np.zeros((32, NSUP * 128), np.float64)
    for s in range(NSUP):
        for bb in range(8):
            for a in range(A):
                w0b[8 * s + bb, s * 128 + 16 * bb + a] = 1.0

    # ln_b projection biases (pre-scan, via ones-row matmul; zero in setup)
    ebias = np.zeros((1, 2 * 128), np.float64)
    for F in range(2):
        bF = c1 * alpha * ((ws[0] - ws[1]) @ ln_b if F == 0 else ws[1] @ ln_b)
        ebias[0, 128 * F:128 * (F + 1)] = np.tile(bF, 8)
    smbias = np.zeros((1, 64), np.float64)
    smbias[0, 0:32] = alpha * (pos @ ln_b)
    smbias[0, 32:64] = alpha * ((gw[0] - gw[1]) @ ln_b)
    co["has_proj_bias"] = bool(np.any(ebias != 0) or np.any(smbias != 0))
    # b_err post-scan bias [128, 2F]
    berr = np.zeros((128, 2), np.float64)
    for F in range(2):
        berr[:, F] = np.tile(beF[F], 8)
    co["has_berr"] = bool(np.any(berr != 0))

    co["dgb"] = float(gb[0] - gb[1])
    consts = {nm: arr.astype(np.float16) for nm, arr in dict(
        c_we=we, c_mue=mue, c_wsm=wsm, c_wsq=wsq, c_musm=musm, c_binv=binv,
        c_binv64=binv64, c_eph=eph, c_w0b=w0b).items()}
    consts.update({nm: arr.astype(np.float32) for nm, arr in dict(
        c_berr=berr, c_ebias=ebias, c_smbias=smbias).items()})
    return co, consts


def _act_table_pass(self):
    """Greedy longest-run activation-table placement (replaces the stock
    Bacc pass): at each activation not covered by the current set, pick the
    set covering the longest upcoming run.  Our program is a single
    straight-line block, so a linear scan is exact."""
    from concourse.hw_specs import get_activation_tables
    tables = list(get_activation_tables(self.m.arch).items())
    for blk in self.main_func.blocks:
        acts = [(idx, inst) for idx, inst in enumerate(blk.instructions)
                if isinstance(inst, mybir.InstActivation)]
        if not acts:
            continue
        funcs = [inst.func for _, inst in acts]
        insertions = []
        i, cur = 0, None
        while i < len(acts):
            if cur is not None and funcs[i] in tables[cur][1]:
                i += 1
                continue
            best, best_len = None, -1
            for sid, (_, fs) in enumerate(tables):
                if funcs[i] not in fs:
                    continue
                ln = 0
                while i + ln < len(funcs) and funcs[i + ln] in fs:
                    ln += 1
                if ln > best_len:
                    best_len, best = ln, sid
            assert best is not None, f"no act set covers {funcs[i]}"
            insertions.append((acts[i][0], best))
            cur = best
            i += best_len
        for idx, sid in reversed(insertions):
            ld = mybir.InstLoadActFuncSet(
                name=self.get_next_instruction_name(),
                act_func_set_id=sid, ins=[], outs=[])
            ld.engine = mybir.EngineType.Activation
            self.register_instruction(ld)
            blk.instructions.insert(idx, ld)


def build_program(nc, co, t_total=T_FULL, ts=512, debug_taps=False, reps=1):
    nsb = t_total // ts
    nch = t_total // R
    ncol = R * (nch + 1)
    ntc = ts // 128                 # 128-row t-chunks per superblock

    import types
    nc.insert_act_table_loads = types.MethodType(_act_table_pass, nc)
    x_in = nc.dram_tensor("x", [t_total, BL, N], F32, kind="ExternalInput").ap()
    out_d = nc.dram_tensor("out", [t_total, BL, A], F32, kind="ExternalOutput").ap()
    shapes = dict(c_we=(128, 2 * 4 * 128), c_mue=(32, NSUP * 2 * 128),
                  c_wsm=(128, 16 * 96), c_wsq=(128, 16 * 32),
                  c_musm=(32, 64), c_binv=(32, NSUP * 128),
                  c_binv64=(32, 64),
                  c_eph=(64, NSUP * 2 * 128), c_w0b=(32, NSUP * 128),
                  c_berr=(128, 2), c_ebias=(1, 2 * 128), c_smbias=(1, 64))
    f32set = {"c_berr", "c_ebias", "c_smbias"}
    cw = {nm: nc.dram_tensor(nm, list(sh), F32 if nm in f32set else F16,
                             kind="ExternalInput").ap()
          for nm, sh in shapes.items()}
    for _rep in range(reps):
        _build_body(nc, co, x_in, out_d, cw, t_total, ts, nsb, nch, ncol, ntc)
    return nc


def _build_body(nc, co, x_in, out_d, cw, t_total, ts, nsb, nch, ncol, ntc):
    with tile.TileContext(nc) as tc, ExitStack() as top:
        consts = top.enter_context(tc.tile_pool(name="consts", bufs=1))
        carry = top.enter_context(tc.tile_pool(name="carry", bufs=1))
        bigp = top.enter_context(tc.tile_pool(name="big", bufs=1))

        ct = {}
        for nm, ap in cw.items():
            t = consts.tile(list(ap.shape), ap.dtype, tag=nm)
            nc.sync.dma_start(out=t, in_=ap)
            ct[nm] = t
        ident = consts.tile([128, 128], F32)
        make_identity(nc, ident)
        eps_col = consts.tile([32, 1], F32); nc.vector.memset(eps_col, LN_EPS)
        halfpi = consts.tile([32, 1], F32); nc.vector.memset(halfpi, math.pi / 2)
        lam_col = consts.tile([128, 1], F32); nc.vector.memset(lam_col, co["lam"])
        lam2_col = consts.tile([128, 1], F32); nc.vector.memset(lam2_col, co["lam2"])
        one_col = consts.tile([128, 1], F32); nc.vector.memset(one_col, 1.0)
        dgb_col = consts.tile([64, 1], F32); nc.vector.memset(dgb_col, 0.5 * co["dgb"])
        ones_row = consts.tile([1, ts], F32); nc.vector.memset(ones_row, 1.0)

        # carries: (s, F) indexed as 2*s + F
        c_ez = carry.tile([128, 2 * NSUP], F32); nc.vector.memset(c_ez, 0.0)
        c_si = carry.tile([128, 2 * NSUP], F32); nc.vector.memset(c_si, 0.0)
        c_ep = carry.tile([128, 2 * NSUP], F32); nc.vector.memset(c_ep, 0.0)
        c_sm = carry.tile([64, 1], F32); nc.vector.memset(c_sm, 0.0)
        c_phi = carry.tile([32, 1], F32); nc.vector.memset(c_phi, 0.0)

        # C time buffer (head zero-padded for chunk-0 warm-up), fp16
        ca = bigp.tile([128, NSUP, ncol], F16)
        for g in range(NSUP):
            nc.vector.memset(ca[:, g, 0:R], 0.0)

        # ================= streaming phase =================
        with ExitStack() as pha:
            p_x = pha.enter_context(tc.tile_pool(name="px", bufs=4))
            p_xt = pha.enter_context(tc.tile_pool(name="pxt", bufs=18))
            p_sq = pha.enter_context(tc.tile_pool(name="psq", bufs=3))
            p_st = pha.enter_context(tc.tile_pool(name="pst", bufs=1))
            p_en = pha.enter_context(tc.tile_pool(name="pen", bufs=2))
            p_ez = pha.enter_context(tc.tile_pool(name="pez", bufs=2))
            p_eh = pha.enter_context(tc.tile_pool(name="peh", bufs=2))
            p_si = pha.enter_context(tc.tile_pool(name="psi", bufs=2))
            p_yk = pha.enter_context(tc.tile_pool(name="pyk", bufs=2))
            p_sm = pha.enter_context(tc.tile_pool(name="psm", bufs=1))
            ps_tp = pha.enter_context(tc.tile_pool(name="pstp", bufs=2, space="PSUM"))
            ps_pe = pha.enter_context(tc.tile_pool(name="pspe", bufs=2, space="PSUM"))
            ps_sm = pha.enter_context(tc.tile_pool(name="pssm", bufs=1, space="PSUM"))
            ps_sml = pha.enter_context(tc.tile_pool(name="pssml", bufs=2, space="PSUM"))

            for sb in range(nsb):
                t0 = sb * ts
                # ---- load ----
                xch = []
                for c in range(ntc):
                    xt = p_x.tile([128, BL * N], F32, tag="xch")
                    nc.sync.dma_start(
                        out=xt,
                        in_=x_in[t0 + 128 * c: t0 + 128 * (c + 1)]
                        .rearrange("t b n -> t (b n)"))
                    xch.append(xt)
                # ---- transpose + copies + smalls/stats chains ----
                smp = ps_sm.tile([128, ts], F32, tag="smp")
                sqp = ps_sm.tile([32, ts], F32, tag="sqp")
                xTs = []
                for p in range(16):
                    tpt = ps_tp.tile([128, ts], F32, tag="tp")
                    for c in range(ntc):
                        nc.tensor.transpose(
                            tpt[:, 128 * c: 128 * (c + 1)],
                            xch[c][:, 128 * p: 128 * (p + 1)], ident)
                    xT = p_xt.tile([128, ts], F16, tag="xnT")
                    nc.scalar.copy(out=xT, in_=tpt)
                    xTs.append(xT)
                    sqT = p_sq.tile([128, ts], F16, tag="sqT")
                    nc.vector.tensor_tensor(out=sqT, in0=xT, in1=xT, op=OP.mult)
                    nc.tensor.matmul(
                        out=smp[0:96, :],
                        lhsT=ct["c_wsm"][:, p * 96:(p + 1) * 96],
                        rhs=xT, start=(p == 0), stop=(p == 15))
                    nc.tensor.matmul(
                        out=sqp, lhsT=ct["c_wsq"][:, p * 32:(p + 1) * 32],
                        rhs=sqT, start=(p == 0), stop=(p == 15))
                if co["has_proj_bias"]:
                    nc.tensor.matmul(out=smp[0:64, :], lhsT=ct["c_smbias"],
                                     rhs=ones_row, start=False, stop=True,
                                     skip_group_check=True)
                # ---- stats ----
                mu_h = p_st.tile([32, ts], F16, tag="mu_h")
                nc.scalar.copy(out=mu_h, in_=smp[64:96, :])
                msq = p_st.tile([32, ts], F32, tag="msq")
                nc.scalar.activation(out=msq, in_=smp[64:96, :],
                                     func=AF.Square)
                varr = p_st.tile([32, ts], F32, tag="varr")
                nc.vector.scalar_tensor_tensor(
                    out=varr, in0=msq, scalar=-1.0, in1=sqp,
                    op0=OP.mult, op1=OP.add)
                sqv = p_st.tile([32, ts], F32, tag="sqv")
                nc.scalar.activation(out=sqv, in_=varr, func=AF.Sqrt,
                                     bias=eps_col)
                invf = p_st.tile([32, ts], F32, tag="invf")
                nc.vector.reciprocal(out=invf, in_=sqv)
                inv_h = p_st.tile([32, ts], F16, tag="inv_h")
                nc.scalar.copy(out=inv_h, in_=invf)
                # smalls mu-correction + inv
                nc.tensor.matmul(out=smp[0:64, :], lhsT=ct["c_musm"],
                                 rhs=mu_h, start=False, stop=True,
                                 skip_group_check=True)
                i64p = ps_sml.tile([128, ts], F32, tag="ib")
                nc.tensor.matmul(out=i64p[0:64, :], lhsT=ct["c_binv64"],
                                 rhs=inv_h, start=True, stop=True)
                i64 = p_st.tile([64, ts], F16, tag="i64s")
                nc.scalar.copy(out=i64, in_=i64p[0:64, :])
                smn = p_sm.tile([64, ts], F16, tag="smn")
                nc.vector.tensor_tensor(out=smn, in0=smp[0:64, :], in1=i64,
                                        op=OP.mult)
                # ---- smalls pipeline ----
                sms = p_sm.tile([64, ts], F16, tag="sms")
                nc.vector.tensor_tensor_scan(
                    out=sms, data0=lam_col[0:64].broadcast_to([64, ts]),
                    data1=smn, initial=c_sm, op0=OP.mult, op1=OP.add)
                nc.gpsimd.tensor_copy(out=c_sm, in_=sms[:, ts - 1:ts])
                sigp = p_sm.tile([32, ts], F32, tag="sigp")
                nc.scalar.activation(out=sigp, in_=sms[0:32, :], func=AF.Tanh)
                om = p_sm.tile([32, ts], F32, tag="om")
                nc.gpsimd.tensor_scalar(out=om, in0=sigp, scalar1=0.02,
                                        scalar2=co["omega_base"],
                                        op0=OP.mult, op1=OP.add)
                nc.gpsimd.tensor_scalar(out=om, in0=om, scalar1=1.0,
                                        scalar2=0.001, op0=OP.min, op1=OP.max)
                phr = p_sm.tile([32, ts], F32, tag="phr")
                nc.vector.tensor_tensor_scan(
                    out=phr, data0=one_col[0:32].broadcast_to([32, ts]),
                    data1=om, initial=c_phi, op0=OP.mult, op1=OP.add)
                # wrap to [-pi, pi] via round-to-nearest int convert
                sc = p_sm.tile([64, ts], F16, tag="sc")
                for (half, ofs, bias) in ((0, 0.0, None), (1, 0.25, halfpi)):
                    wf = p_sm.tile([32, ts], F32, tag=f"wf{half}")
                    if ofs:
                        nc.gpsimd.tensor_scalar(
                            out=wf, in0=phr, scalar1=(1.0 / TWO_PI),
                            scalar2=ofs, op0=OP.mult, op1=OP.add)
                    else:
                        nc.gpsimd.tensor_scalar(
                            out=wf, in0=phr, scalar1=(1.0 / TWO_PI),
                            scalar2=None, op0=OP.mult)
                    wi = p_sm.tile([32, ts], I32, tag=f"wi{half}")
                    nc.vector.tensor_copy(out=wi, in_=wf)
                    nc.vector.tensor_copy(out=wf, in_=wi)
                    wrap = p_sm.tile([32, ts], F32, tag=f"wrap{half}")
                    nc.vector.scalar_tensor_tensor(
                        out=wrap, in0=wf, scalar=-TWO_PI, in1=phr,
                        op0=OP.mult, op1=OP.add)
                    if bias is None:
                        nc.scalar.activation(out=sc[0:32, :], in_=wrap,
                                             func=AF.Sin)
                    else:
                        nc.scalar.activation(out=sc[32:64, :], in_=wrap,
                                             func=AF.Sin, bias=bias)
                # carry: c_phi = wrapped phr last column
                cwf = p_sm.tile([32, 1], F32, tag="cwf")
                cwi = p_sm.tile([32, 1], I32, tag="cwi")
                nc.vector.tensor_scalar(out=cwf, in0=phr[:, ts - 1:ts],
                                        scalar1=(1.0 / TWO_PI),
                                        scalar2=None, op0=OP.mult)
                nc.vector.tensor_copy(out=cwi, in_=cwf)
                nc.vector.tensor_copy(out=cwf, in_=cwi)
                nc.vector.scalar_tensor_tensor(out=c_phi, in0=cwf, scalar=-TWO_PI,
                                               in1=phr[:, ts - 1:ts],
                                               op0=OP.mult, op1=OP.add)
                wh = p_sm.tile([32, ts], F16, tag="wh")
                nc.scalar.activation(out=wh, in_=sms[32:64, :], func=AF.Tanh,
                                     scale=0.5, bias=dgb_col[32:64])
                w0 = p_sm.tile([32, ts], F16, tag="w0")
                nc.gpsimd.tensor_scalar(out=w0, in0=wh, scalar1=0.5,
                                        scalar2=0.5, op0=OP.mult, op1=OP.add)

                # ---- per-superset feature pipeline ----
                for s in range(NSUP):
                    pe = [ps_pe.tile([128, ts], F32, tag="pe", name=f"pe{F}")
                          for F in range(2)]
                    for F in range(2):
                        for pw in range(4):
                            p = 4 * s + pw
                            nc.tensor.matmul(
                                out=pe[F],
                                lhsT=ct["c_we"][:, (F * 4 + pw) * 128:
                                                (F * 4 + pw + 1) * 128],
                                rhs=xTs[p], start=(pw == 0), stop=(pw == 3))
                        if co["has_proj_bias"]:
                            nc.tensor.matmul(
                                out=pe[F],
                                lhsT=ct["c_ebias"][:, 128 * F:128 * (F + 1)],
                                rhs=ones_row, start=False, stop=True,
                                skip_group_check=True)
                        nc.tensor.matmul(
                            out=pe[F],
                            lhsT=ct["c_mue"][:, 128 * (s * 2 + F):
                                             128 * (s * 2 + F + 1)],
                            rhs=mu_h, start=False, stop=True,
                            skip_group_check=True)
                    ibp = ps_sml.tile([128, ts], F32, tag="ib")
                    nc.tensor.matmul(out=ibp,
                                     lhsT=ct["c_binv"][:, 128 * s:128 * (s + 1)],
                                     rhs=inv_h, start=True, stop=True)
                    ib = p_st.tile([128, ts], F16, tag="ibs")
                    nc.scalar.copy(out=ib, in_=ibp)
                    peh = p_en.tile([128, 2, ts], F16, tag="peh")
                    for F in range(2):
                        nc.scalar.copy(out=peh[:, F, :], in_=pe[F])
                    en = p_en.tile([128, 2, ts], F16, tag="en")
                    nc.vector.tensor_tensor(
                        out=en, in0=peh,
                        in1=ib[:, None, :].broadcast_to([128, 2, ts]),
                        op=OP.mult)
                    ez = p_ez.tile([128, 2, ts], F16, tag="ez")
                    for F in range(2):
                        sk = 2 * s + F
                        nc.vector.tensor_tensor_scan(
                            out=ez[:, F, :],
                            data0=lam_col.broadcast_to([128, ts]),
                            data1=en[:, F, :], initial=c_ez[:, sk:sk + 1],
                            op0=OP.mult, op1=OP.add)
                        nc.gpsimd.tensor_copy(out=c_ez[:, sk:sk + 1],
                                              in_=ez[:, F, ts - 1:ts])
                    # phase features + e assembly
                    ep = [ps_pe.tile([128, ts], F32, tag="pe", name=f"ep{F}")
                          for F in range(2)]
                    eh = p_eh.tile([128, 2, ts + 1], F16, tag="eh")
                    nc.gpsimd.tensor_copy(out=eh[:, :, 0:1],
                                          in_=c_ep[:, 2 * s:2 * s + 2, None])
                    ephh = p_eh.tile([128, 2, ts], F16, tag="ephh")
                    for F in range(2):
                        nc.tensor.matmul(
                            out=ep[F],
                            lhsT=ct["c_eph"][:, 128 * (s * 2 + F):
                                             128 * (s * 2 + F + 1)],
                            rhs=sc, start=True, stop=True)
                        if co["has_berr"]:
                            nc.vector.scalar_tensor_tensor(
                                out=eh[:, F, 1:ts + 1], in0=ep[F],
                                scalar=ct["c_berr"][:, F:F + 1],
                                in1=ez[:, F, :], op0=OP.add, op1=OP.add)
                        else:
                            nc.scalar.copy(out=ephh[:, F, :], in_=ep[F])
                    if not co["has_berr"]:
                        nc.vector.tensor_tensor(
                            out=eh[:, :, 1:ts + 1], in0=ephh, in1=ez,
                            op=OP.add)
                    nc.gpsimd.tensor_copy(
                        out=c_ep[:, 2 * s:2 * s + 2, None],
                        in_=eh[:, :, ts:ts + 1])
                    si = p_si.tile([128, 2, ts], F16, tag="si")
                    for F in range(2):
                        sk = 2 * s + F
                        nc.vector.tensor_tensor_scan(
                            out=si[:, F, :],
                            data0=lam2_col.broadcast_to([128, ts]),
                            data1=eh[:, F, 1:ts + 1],
                            initial=c_si[:, sk:sk + 1],
                            op0=OP.mult, op1=OP.add)
                        nc.gpsimd.tensor_copy(out=c_si[:, sk:sk + 1],
                                              in_=si[:, F, ts - 1:ts])
                    t1s = p_yk.tile([128, 2, ts], F16, tag="t1s")
                    nc.gpsimd.tensor_scalar(out=t1s, in0=si,
                                            scalar1=co["ki_c1"], scalar2=None,
                                            op0=OP.mult)
                    yk = p_yk.tile([128, 2, ts], F16, tag="yk")
                    nc.vector.tensor_tensor(out=yk, in0=t1s,
                                            in1=eh[:, :, 1:ts + 1], op=OP.add)
                    t2s = p_yk.tile([128, 2, ts], F16, tag="t2s")
                    nc.gpsimd.tensor_scalar(out=t2s, in0=eh[:, :, 0:ts],
                                            scalar1=-co["kd_c1"], scalar2=None,
                                            op0=OP.mult)
                    nc.vector.tensor_tensor(out=yk, in0=yk, in1=t2s, op=OP.add)
                    # C = ykS + w0 * ykD
                    w0p = ps_sml.tile([128, ts], F32, tag="ib", name="w0p")
                    nc.tensor.matmul(
                        out=w0p, lhsT=ct["c_w0b"][:, 128 * s:128 * (s + 1)],
                        rhs=w0, start=True, stop=True)
                    w0ph = p_st.tile([128, ts], F16, tag="w0ph")
                    nc.scalar.copy(out=w0ph, in_=w0p)
                    tD = p_en.tile([128, ts], F16, tag="tD")
                    nc.vector.tensor_tensor(out=tD, in0=w0ph, in1=yk[:, 0, :],
                                            op=OP.mult)
                    nc.vector.tensor_tensor(
                        out=ca[:, s, R + t0: R + t0 + ts], in0=tD,
                        in1=yk[:, 1, :], op=OP.add)

        # ================= overlap-save sweep =================
        with ExitStack() as phbc:
            paw = phbc.enter_context(tc.tile_pool(name="paw", bufs=1))
            a_wide = paw.tile([128, NSUP, ncol], F32)
            with ExitStack() as phb:
                swp = phb.enter_context(tc.tile_pool(name="swp", bufs=3))
                swp8 = phb.enter_context(tc.tile_pool(name="swp8", bufs=2))
                ca4 = ca.rearrange("p g (c r) -> p g c r", r=R)
                aw4 = a_wide.rearrange("p g (c r) -> p g c r", r=R)
                nh = nch // 2

                def tsl(t4, j, grp):
                    # column j in [R-W, 2R-W) of each chunk's window
                    if j < R:
                        return t4[:, :, grp:nch:2, j]
                    return t4[:, :, 1 + grp:nch + 1:2, j - R]

                for grp in range(2):
                    nc.vector.memset(tsl(aw4, R - W - 1, grp), 0.0)
                # D pre-scaled by kappa; updated once per DP-step block
                # (at the block's first step, effective from the next block;
                # validated: D lag up to 2*DP changes nothing) with
                # geometric-sum coefficients.  tanh(C - D) is batched: u8/h8
                # cover a whole DP block in one TT/Activation pair, computed
                # a block ahead so the per-step chain is only q -> rr -> a'.
                eff = sum(co["lam2"] ** jj for jj in range(DP))
                lam2dp = co["lam2"] ** DP
                nblk = (R + W) // DP
                assert nblk * DP == R + W and (R - W) % DP == 0

                def blk_ca(jb):
                    # merged-group slice [128, NSUP, nch, DP] at cols jb..jb+DP
                    if jb < R:
                        return ca4[:, :, 0:nch, jb:jb + DP]
                    return ca4[:, :, 1:nch + 1, jb - R:jb - R + DP]

                d_prev = [None, None]
                rr_prev = [None, None]
                p_next = {}          # (grp, j) -> precomputed h(j) - a(j-2)
                h8 = None
                h8n = None
                for blk in range(nblk):
                    jb = (R - W) + blk * DP
                    if blk == 0:
                        h8 = swp8.tile([128, NSUP, nch, DP], F32, tag="h8")
                        nc.scalar.activation(out=h8, in_=blk_ca(jb),
                                             func=AF.Tanh)
                    h8_cur, h8_nxt = h8, None
                    for k in range(DP):
                        j = jb + k
                        i = blk * DP + k

                        def hslice(kk, grp):
                            if kk < DP:
                                return h8_cur[:, :, grp::2, kk]
                            return h8_nxt[:, :, grp::2, kk - DP]

                        # loop-carried chain: rr(j-1) -> q(j) -> rr(j).
                        # q first in each engine queue.
                        qs = []
                        for grp in range(2):
                            q = swp.tile([128, NSUP, nh], F32, tag=f"q{grp}")
                            if i < 2:
                                nc.vector.tensor_tensor(
                                    out=q, in0=hslice(k, grp),
                                    in1=tsl(aw4, j - 1, grp), op=OP.subtract)
                            else:
                                nc.vector.scalar_tensor_tensor(
                                    out=q, in0=rr_prev[grp],
                                    scalar=-co["rate"],
                                    in1=p_next.pop((grp, j)),
                                    op0=OP.mult, op1=OP.add)
                            qs.append(q)
                        rrs = []
                        for grp in range(2):
                            rr = swp.tile([128, NSUP, nh], F32, tag=f"r{grp}")
                            nc.scalar.activation(out=rr, in_=qs[grp],
                                                 func=AF.Tanh, scale=co["s2"])
                            rrs.append(rr)
                            rr_prev[grp] = rr
                        for grp in range(2):
                            nc.vector.scalar_tensor_tensor(
                                out=tsl(aw4, j, grp), in0=rrs[grp],
                                scalar=co["rate"], in1=tsl(aw4, j - 1, grp),
                                op0=OP.mult, op1=OP.add)
                        # off-chain: p(j+2) = h(j+2) - a(j), on Pool
                        if i + 2 < R + W:
                            for grp in range(2):
                                p = swp.tile([128, NSUP, nh], F32,
                                             tag=f"p{grp}")
                                nc.gpsimd.tensor_tensor(
                                    out=p, in0=hslice(k + 2, grp),
                                    in1=tsl(aw4, j, grp), op=OP.subtract)
                                p_next[(grp, j + 2)] = p
                        if k == 0 and blk < nblk - 1:
                            # D update from this step's (q, rr); u8/h8 for the
                            # next block (split across Pool/DVE; 7 steps of
                            # slack to hide it)
                            for grp in range(2):
                                tq = swp.tile([128, NSUP, nh], F32,
                                              tag=f"tq{grp}")
                                nc.gpsimd.tensor_scalar(
                                    out=tq, in0=qs[grp],
                                    scalar1=-eff * co["kb"],
                                    scalar2=None, op0=OP.mult)
                                t1 = swp.tile([128, NSUP, nh], F32,
                                              tag=f"t1{grp}")
                                nc.vector.scalar_tensor_tensor(
                                    out=t1, in0=rrs[grp],
                                    scalar=eff * co["kr"],
                                    in1=tq, op0=OP.mult, op1=OP.add)
                                if d_prev[grp] is None:
                                    d_prev[grp] = t1
                                else:
                                    d_new = swp.tile([128, NSUP, nh], F32,
                                                     tag=f"dn{grp}")
                                    nc.vector.scalar_tensor_tensor(
                                        out=d_new, in0=d_prev[grp],
                                        scalar=lam2dp, in1=t1,
                                        op0=OP.mult, op1=OP.add)
                                    d_prev[grp] = d_new
                            njb = jb + DP
                            u8 = swp8.tile([128, NSUP, nch, DP], F32,
                                           tag="u8")
                            cblk = blk_ca(njb)
                            for grp in range(2):
                                eng = nc.gpsimd if grp == 0 else nc.vector
                                eng.tensor_tensor(
                                    out=u8[:, :, grp::2, :],
                                    in0=cblk[:, :, grp::2, :],
                                    in1=d_prev[grp][:, :, :, None]
                                    .broadcast_to([128, NSUP, nh, DP]),
                                    op=OP.subtract)
                            h8n = swp8.tile([128, NSUP, nch, DP], F32,
                                            tag="h8", name="h8n")
                            nc.scalar.activation(out=h8n, in_=u8,
                                                 func=AF.Tanh)
                            h8_nxt = h8n
                    h8 = h8_nxt if h8_nxt is not None else h8_cur

            # ============= output transpose + store =============
            with ExitStack() as phc:
                ps_o = phc.enter_context(tc.tile_pool(name="pso", bufs=2,
                                                      space="PSUM"))
                p_o = phc.enter_context(tc.tile_pool(name="po", bufs=3))
                for tau in range(t_total // 128):
                    tp = ps_o.tile([128, NSUP, 128], F32, tag="otp")
                    for g in range(NSUP):
                        nc.tensor.transpose(
                            tp[:, g, :],
                            a_wide[:, g, R + 128 * tau: R + 128 * (tau + 1)],
                            ident)
                    ot = p_o.tile([128, NSUP * 128], F32, tag="ot")
                    if tau % 2 == 0:
                        nc.scalar.copy(out=ot,
                                       in_=tp.rearrange("p g c -> p (g c)"))
                    else:
                        nc.vector.tensor_copy(
                            out=ot, in_=tp.rearrange("p g c -> p (g c)"))
                    nc.sync.dma_start(
                        out=out_d[128 * tau: 128 * (tau + 1)]
                        .rearrange("t b a -> t (b a)"), in_=ot)
    return nc


def _in_maps(inputs, consts):
    x = np.ascontiguousarray(np.asarray(inputs["states"], np.float32))
    maps = []
    for j in range(NCORES):
        m = {"x": np.ascontiguousarray(x[:, BL * j: BL * (j + 1), :])}
        m.update(consts)
        maps.append(m)
    return maps


def kernel(**inputs):
    co, consts = _coeffs(inputs)
    nc = bacc.Bacc("TRN2", num_devices=NCORES)
    build_program(nc, co)
    nc.compile()
    maps = _in_maps(inputs, consts)
    res = run_bass_kernel_spmd(nc, maps, list(range(NCORES)))
    outs = [np.asarray(res.results[j]["out"]).reshape(T_FULL, BL, A)
            for j in range(NCORES)]
    return np.concatenate(outs, axis=1)


# revision 37
# speedup vs baseline: 5.1216x; 5.1216x over previous
"""Trainium2 Bass kernel for nn_ControlPolicy (T=4096, B=256, N=64, K=2, A=16).

Sharding: data-parallel over batch B across 8 NeuronCores (32 rows/core);
tiny parameters replicated.

v2 design notes (per core):
  LayerNorm is never applied to the full [T,B,N] tensor.  Raw x is
  transposed (PE) and projected (PE matmuls); per-(t,b) mean and E[x^2]
  ride along as extra rows of the smalls matmul chain (ones/64 lhsT
  columns).  Normalization is applied to the 16x smaller projected
  features: e_norm = (W@x - mu*(W@1)) * inv, with the mu-term folded in as
  one more matmul on the accumulation chain and inv broadcast across
  (b,a)-rows by a PE matmul.

  K is collapsed before the scans: the projection emits D = e0-e1 and
  S = e1 feature sets (lhsT rows differenced on the host); the PID chain
  (alpha-filter scan, integrator scan, kp/ki/kd combination) is linear, so
  C = ykS + w0 * ykD.

  The only nonlinear recurrence (a, D-state) runs as an overlap-save
  chunked sweep (R=64, W=16 warm-up, validated ~1.4e-3 rel): all 64 chunks
  advance together, split into even/odd groups for two independent
  dependency chains.  The anti-windup D-state is small (|kappa*D|<=0.01)
  and slow (lam2=0.88), so it is only updated once per DP=8-step block
  with geometric-sum-corrected coefficients (validated: no error change),
  and tanh(C - D) is batched: one TT + one Activation cover a whole block
  for all chunks, precomputed a block ahead.  The per-step loop-carried
  chain is only rr -> q -> rr via q(j) = p(j) - rate*rr(j-1) with
  p(j) = h(j) - a(j-2) materialized two steps early on the Pool engine.
"""
import math
import numpy as np
from contextlib import ExitStack

import concourse.bass as bass
import concourse.bacc as bacc
import concourse.tile as tile
from concourse import mybir
from concourse.bass_utils import run_bass_kernel_spmd
from concourse.masks import make_identity

F32 = mybir.dt.float32
F16 = mybir.dt.float16
I32 = mybir.dt.int32
OP = mybir.AluOpType
AF = mybir.ActivationFunctionType
AX = mybir.AxisListType

T_FULL = 4096
B_FULL = 256
N = 64
K = 2
A = 16
NCORES = 8
BL = B_FULL // NCORES          # 32
LN_EPS = 1e-5
TWO_PI = float(np.float32(2.0 * np.pi))

R = 64                          # sweep chunk length
W = 16                          # sweep warm-up
DP = 8                          # sweep D-state update period
NSUP = 4                        # supersets of 8 b-rows


def _sigmoid(x): return 1.0 / (1.0 + math.exp(-x))
def _softplus(x): return math.log1p(math.exp(x))


def _coeffs(inputs):
    f = lambda k: float(np.asarray(inputs[k], np.float64))
    alpha = _sigmoid(f("filter_alpha_logit"))
    leak = _sigmoid(f("int_leak_logit"))
    beta = _sigmoid(f("act_beta_logit"))
    rate = 0.25 * _sigmoid(f("rate_limit_raw"))
    aw = _softplus(f("aw_gain_raw"))
    omega_base = _softplus(f("phase_omega_raw")) + 0.001

    kp_a = np.log1p(np.exp(np.asarray(inputs["kp_raw"], np.float64)))
    ki_a = np.log1p(np.exp(np.asarray(inputs["ki_raw"], np.float64)))
    kd_a = np.log1p(np.exp(np.asarray(inputs["kd_raw"], np.float64)))
    for nm, arr in (("kp", kp_a), ("ki", ki_a), ("kd", kd_a)):
        assert np.allclose(arr, arr.flat[0], rtol=1e-12), f"{nm} not uniform"
    kp, ki, kd = float(kp_a.flat[0]), float(ki_a.flat[0]), float(kd_a.flat[0])

    lam2 = 1.0 - leak
    c1 = kp + kd
    kappa = ki * aw * lam2
    co = dict(
        alpha=alpha, lam=1.0 - alpha, lam2=lam2, beta=beta, rate=rate,
        omega_base=omega_base, c1=c1, kappa=kappa,
        s2=beta / (rate + 1e-6), ki_c1=ki / c1, kd_c1=kd / c1,
        kr=kappa * rate, kb=kappa * beta,
    )

    ln_w = np.asarray(inputs["ln_w"], np.float64)
    ln_b = np.asarray(inputs["ln_b"], np.float64)
    ws = np.asarray(inputs["w_state"], np.float64)
    wp = np.asarray(inputs["w_phase"], np.float64)
    b_err = np.asarray(inputs["b_err"], np.float64)
    gw = np.asarray(inputs["gate_w"], np.float64)
    gb = np.asarray(inputs["gate_b"], np.float64)
    pos = np.asarray(inputs["phase_omega_state"], np.float64)

    # feature row sets: D = k0 - k1, S = k1
    WF = [c1 * alpha * (ws[0] - ws[1]) * ln_w,      # [A, N] each
          c1 * alpha * ws[1] * ln_w]
    wpos = alpha * pos * ln_w                        # [N]
    wgd = alpha * (gw[0] - gw[1]) * ln_w
    wpF = [c1 * (wp[0] - wp[1]), c1 * wp[1]]         # [A, 2]
    beF = [c1 * (b_err[0] - b_err[1]), c1 * b_err[1]]

    # fp16-cast weights; row sums for the mu-correction computed from the
    # cast values so the correction matches the matmul exactly
    WFh = [w.astype(np.float16) for w in WF]
    wposh = wpos.astype(np.float16)
    wgdh = wgd.astype(np.float16)
    p1F = [w.astype(np.float64).sum(axis=1) for w in WFh]   # [A] each
    p1pos = float(wposh.astype(np.float64).sum())
    p1gd = float(wgdh.astype(np.float64).sum())

    # e-projection lhsT [128, 2F*4pw*128]: block (F, pw) maps pair pw's
    # partitions (bw2, n64) onto out rows (b8, a16) = 16*(2*pw+bw) + a
    we = np.zeros((128, 2 * 4 * 128), np.float64)
    for F in range(2):
        for pw in range(4):
            base = (F * 4 + pw) * 128
            for bw in range(2):
                for a in range(A):
                    we[bw * 64:(bw + 1) * 64,
                       base + 16 * (2 * pw + bw) + a] = WFh[F][a, :]
    # mu-correction lhsT [32, 4s*2F*128]: rhs = mu_h (full 32 partitions)
    mue = np.zeros((32, NSUP * 2 * 128), np.float64)
    for s in range(NSUP):
        for F in range(2):
            base = (s * 2 + F) * 128
            for bb in range(8):
                for a in range(A):
                    mue[8 * s + bb, base + 16 * bb + a] = -p1F[F][a]
    # smalls+stats chain lhsT [128, 16p*96]: out rows 0-31 pos, 32-63 gdiff,
    # 64-95 mu (ones/64)
    wsm = np.zeros((128, 16 * 96), np.float64)
    for p in range(16):
        for bw in range(2):
            for n in range(N):
                r0 = bw * 64 + n
                wsm[r0, p * 96 + 2 * p + bw] = wposh[n]
                wsm[r0, p * 96 + 32 + 2 * p + bw] = wgdh[n]
                wsm[r0, p * 96 + 64 + 2 * p + bw] = 1.0 / N
    # sq chain lhsT [128, 16p*32]: out rows 96-127 = E[x^2]
    wsq = np.zeros((128, 16 * 32), np.float64)
    for p in range(16):
        for bw in range(2):
            wsq[bw * 64:(bw + 1) * 64, p * 32 + 2 * p + bw] = 1.0 / N
    # smalls mu-correction lhsT [32, 64]: rhs = mu_h
    musm = np.zeros((32, 64), np.float64)
    for b in range(32):
        musm[b, b] = -p1pos
        musm[b, 32 + b] = -p1gd
    # inv broadcast lhsT [32, 4s*128] (rhs = inv_h) and [32, 64]
    binv = np.zeros((32, NSUP * 128), np.float64)
    for s in range(NSUP):
        for bb in range(8):
            binv[8 * s + bb, s * 128 + 16 * bb:s * 128 + 16 * (bb + 1)] = 1.0
    binv64 = np.zeros((32, 64), np.float64)
    for b in range(32):
        binv64[b, b] = 1.0
        binv64[b, 32 + b] = 1.0
    # phase-feature lhsT [64, 4s*2F*128]: rhs = sc (sin rows 0-31, cos 32-63)
    eph = np.zeros((64, NSUP * 2 * 128), np.float64)
    for s in range(NSUP):
        for F in range(2):
            base = (s * 2 + F) * 128
            for bb in range(8):
                for a in range(A):
                    eph[8 * s + bb, base + 16 * bb + a] = wpF[F][a, 0]
                    eph[32 + 8 * s + bb, base + 16 * bb + a] = wpF[F][a, 1]
    # w0 broadcast lhsT [32, 4s*128]
    w0b = np.zeros((32, NSUP * 128), np.float64)
    for s in range(NSUP):
        for bb in range(8):
            for a in range(A):
                w0b[8 * s + bb, s * 128 + 16 * bb + a] = 1.0

    # ln_b projection biases (pre-scan, via ones-row matmul; zero in setup)
    ebias = np.zeros((1, 2 * 128), np.float64)
    for F in range(2):
        bF = c1 * alpha * ((ws[0] - ws[1]) @ ln_b if F == 0 else ws[1] @ ln_b)
        ebias[0, 128 * F:128 * (F + 1)] = np.tile(bF, 8)
    smbias = np.zeros((1, 64), np.float64)
    smbias[0, 0:32] = alpha * (pos @ ln_b)
    smbias[0, 32:64] = alpha * ((gw[0] - gw[1]) @ ln_b)
    co["has_proj_bias"] = bool(np.any(ebias != 0) or np.any(smbias != 0))
    # b_err post-scan bias [128, 2F]
    berr = np.zeros((128, 2), np.float64)
    for F in range(2):
        berr[:, F] = np.tile(beF[F], 8)
    co["has_berr"] = bool(np.any(berr != 0))

    co["dgb"] = float(gb[0] - gb[1])
    consts = {nm: arr.astype(np.float16) for nm, arr in dict(
        c_we=we, c_mue=mue, c_wsm=wsm, c_wsq=wsq, c_musm=musm, c_binv=binv,
        c_binv64=binv64, c_eph=eph, c_w0b=w0b).items()}
    consts.update({nm: arr.astype(np.float32) for nm, arr in dict(
        c_berr=berr, c_ebias=ebias, c_smbias=smbias).items()})
    return co, consts


def _act_table_pass(self):
    """Greedy longest-run activation-table placement (replaces the stock
    Bacc pass): at each activation not covered by the current set, pick the
    set covering the longest upcoming run.  Our program is a single
    straight-line block, so a linear scan is exact."""
    from concourse.hw_specs import get_activation_tables
    tables = list(get_activation_tables(self.m.arch).items())
    for blk in self.main_func.blocks:
        acts = [(idx, inst) for idx, inst in enumerate(blk.instructions)
                if isinstance(inst, mybir.InstActivation)]
        if not acts:
            continue
        funcs = [inst.func for _, inst in acts]
        insertions = []
        i, cur = 0, None
        while i < len(acts):
            if cur is not None and funcs[i] in tables[cur][1]:
                i += 1
                continue
            best, best_len = None, -1
            for sid, (_, fs) in enumerate(tables):
                if funcs[i] not in fs:
                    continue
                ln = 0
                while i + ln < len(funcs) and funcs[i + ln] in fs:
                    ln += 1
                if ln > best_len:
                    best_len, best = ln, sid
            assert best is not None, f"no act set covers {funcs[i]}"
            insertions.append((acts[i][0], best))
            cur = best
            i += best_len
        for idx, sid in reversed(insertions):
            ld = mybir.InstLoadActFuncSet(
                name=self.get_next_instruction_name(),
                act_func_set_id=sid, ins=[], outs=[])
            ld.engine = mybir.EngineType.Activation
            self.register_instruction(ld)
            blk.instructions.insert(idx, ld)


def build_program(nc, co, t_total=T_FULL, ts=512, debug_taps=False, reps=1):
    nsb = t_total // ts
    nch = t_total // R
    ncol = R * (nch + 1)
    ntc = ts // 128                 # 128-row t-chunks per superblock

    import types
    nc.insert_act_table_loads = types.MethodType(_act_table_pass, nc)
    x_in = nc.dram_tensor("x", [t_total, BL, N], F32, kind="ExternalInput").ap()
    out_d = nc.dram_tensor("out", [t_total, BL, A], F32, kind="ExternalOutput").ap()
    shapes = dict(c_we=(128, 2 * 4 * 128), c_mue=(32, NSUP * 2 * 128),
                  c_wsm=(128, 16 * 96), c_wsq=(128, 16 * 32),
                  c_musm=(32, 64), c_binv=(32, NSUP * 128),
                  c_binv64=(32, 64),
                  c_eph=(64, NSUP * 2 * 128), c_w0b=(32, NSUP * 128),
                  c_berr=(128, 2), c_ebias=(1, 2 * 128), c_smbias=(1, 64))
    f32set = {"c_berr", "c_ebias", "c_smbias"}
    cw = {nm: nc.dram_tensor(nm, list(sh), F32 if nm in f32set else F16,
                             kind="ExternalInput").ap()
          for nm, sh in shapes.items()}
    for _rep in range(reps):
        _build_body(nc, co, x_in, out_d, cw, t_total, ts, nsb, nch, ncol, ntc)
    return nc


def _build_body(nc, co, x_in, out_d, cw, t_total, ts, nsb, nch, ncol, ntc):
    with tile.TileContext(nc) as tc, ExitStack() as top:
        consts = top.enter_context(tc.tile_pool(name="consts", bufs=1))
        carry = top.enter_context(tc.tile_pool(name="carry", bufs=1))
        bigp = top.enter_context(tc.tile_pool(name="big", bufs=1))

        ct = {}
        for nm, ap in cw.items():
            t = consts.tile(list(ap.shape), ap.dtype, tag=nm)
            nc.sync.dma_start(out=t, in_=ap)
            ct[nm] = t
        ident = consts.tile([128, 128], F32)
        make_identity(nc, ident)
        eps_col = consts.tile([32, 1], F32); nc.vector.memset(eps_col, LN_EPS)
        halfpi = consts.tile([32, 1], F32); nc.vector.memset(halfpi, math.pi / 2)
        lam_col = consts.tile([128, 1], F32); nc.vector.memset(lam_col, co["lam"])
        lam2_col = consts.tile([128, 1], F32); nc.vector.memset(lam2_col, co["lam2"])
        one_col = consts.tile([128, 1], F32); nc.vector.memset(one_col, 1.0)
        dgb_col = consts.tile([64, 1], F32); nc.vector.memset(dgb_col, 0.5 * co["dgb"])
        ones_row = consts.tile([1, ts], F32); nc.vector.memset(ones_row, 1.0)

        # carries: (s, F) indexed as 2*s + F
        c_ez = carry.tile([128, 2 * NSUP], F32); nc.vector.memset(c_ez, 0.0)
        c_si = carry.tile([128, 2 * NSUP], F32); nc.vector.memset(c_si, 0.0)
        c_ep = carry.tile([128, 2 * NSUP], F32); nc.vector.memset(c_ep, 0.0)
        c_sm = carry.tile([64, 1], F32); nc.vector.memset(c_sm, 0.0)
        c_phi = carry.tile([32, 1], F32); nc.vector.memset(c_phi, 0.0)

        # C time buffer (head zero-padded for chunk-0 warm-up), fp16
        ca = bigp.tile([128, NSUP, ncol], F16)
        for g in range(NSUP):
            nc.vector.memset(ca[:, g, 0:R], 0.0)

        # ================= streaming phase =================
        with ExitStack() as pha:
            p_x = pha.enter_context(tc.tile_pool(name="px", bufs=4))
            p_xt = pha.enter_context(tc.tile_pool(name="pxt", bufs=18))
            p_sq = pha.enter_context(tc.tile_pool(name="psq", bufs=3))
            p_st = pha.enter_context(tc.tile_pool(name="pst", bufs=1))
            p_st2 = pha.enter_context(tc.tile_pool(name="pst2", bufs=2))
            p_en = pha.enter_context(tc.tile_pool(name="pen", bufs=2))
            p_ez = pha.enter_context(tc.tile_pool(name="pez", bufs=2))
            p_eh = pha.enter_context(tc.tile_pool(name="peh", bufs=2))
            p_si = pha.enter_context(tc.tile_pool(name="psi", bufs=2))
            p_yk = pha.enter_context(tc.tile_pool(name="pyk", bufs=2))
            p_sm = pha.enter_context(tc.tile_pool(name="psm", bufs=1))
            ps_tp = pha.enter_context(tc.tile_pool(name="pstp", bufs=2, space="PSUM"))
            ps_pe = pha.enter_context(tc.tile_pool(name="pspe", bufs=2, space="PSUM"))
            ps_sm = pha.enter_context(tc.tile_pool(name="pssm", bufs=1, space="PSUM"))
            ps_sml = pha.enter_context(tc.tile_pool(name="pssml", bufs=2, space="PSUM"))

            for sb in range(nsb):
                t0 = sb * ts
                # ---- load ----
                xch = []
                for c in range(ntc):
                    xt = p_x.tile([128, BL * N], F32, tag="xch")
                    nc.sync.dma_start(
                        out=xt,
                        in_=x_in[t0 + 128 * c: t0 + 128 * (c + 1)]
                        .rearrange("t b n -> t (b n)"))
                    xch.append(xt)
                # ---- transpose + copies + smalls/stats chains ----
                smp = ps_sm.tile([128, ts], F32, tag="smp")
                sqp = ps_sm.tile([32, ts], F32, tag="sqp")
                xTs = []
                for p in range(16):
                    tpt = ps_tp.tile([128, ts], F32, tag="tp")
                    for c in range(ntc):
                        nc.tensor.transpose(
                            tpt[:, 128 * c: 128 * (c + 1)],
                            xch[c][:, 128 * p: 128 * (p + 1)], ident)
                    xT = p_xt.tile([128, ts], F16, tag="xnT")
                    nc.scalar.copy(out=xT, in_=tpt)
                    xTs.append(xT)
                    sqT = p_sq.tile([128, ts], F16, tag="sqT")
                    nc.vector.tensor_tensor(out=sqT, in0=xT, in1=xT, op=OP.mult)
                    nc.tensor.matmul(
                        out=smp[0:96, :],
                        lhsT=ct["c_wsm"][:, p * 96:(p + 1) * 96],
                        rhs=xT, start=(p == 0), stop=(p == 15))
                    nc.tensor.matmul(
                        out=sqp, lhsT=ct["c_wsq"][:, p * 32:(p + 1) * 32],
                        rhs=sqT, start=(p == 0), stop=(p == 15))
                if co["has_proj_bias"]:
                    nc.tensor.matmul(out=smp[0:64, :], lhsT=ct["c_smbias"],
                                     rhs=ones_row, start=False, stop=True,
                                     skip_group_check=True)
                # ---- stats ----
                mu_h = p_st2.tile([32, ts], F16, tag="mu_h")
                nc.scalar.copy(out=mu_h, in_=smp[64:96, :])
                msq = p_st.tile([32, ts], F32, tag="msq")
                nc.scalar.activation(out=msq, in_=smp[64:96, :],
                                     func=AF.Square)
                varr = p_st.tile([32, ts], F32, tag="varr")
                nc.vector.scalar_tensor_tensor(
                    out=varr, in0=msq, scalar=-1.0, in1=sqp,
                    op0=OP.mult, op1=OP.add)
                sqv = p_st.tile([32, ts], F32, tag="sqv")
                nc.scalar.activation(out=sqv, in_=varr, func=AF.Sqrt,
                                     bias=eps_col)
                invf = p_st.tile([32, ts], F32, tag="invf")
                nc.vector.reciprocal(out=invf, in_=sqv)
                inv_h = p_st2.tile([32, ts], F16, tag="inv_h")
                nc.scalar.copy(out=inv_h, in_=invf)
                # smalls mu-correction + inv
                nc.tensor.matmul(out=smp[0:64, :], lhsT=ct["c_musm"],
                                 rhs=mu_h, start=False, stop=True,
                                 skip_group_check=True)
                i64p = ps_sml.tile([128, ts], F32, tag="ib")
                nc.tensor.matmul(out=i64p[0:64, :], lhsT=ct["c_binv64"],
                                 rhs=inv_h, start=True, stop=True)
                i64 = p_st.tile([64, ts], F16, tag="i64s")
                nc.scalar.copy(out=i64, in_=i64p[0:64, :])
                smn = p_sm.tile([64, ts], F16, tag="smn")
                nc.vector.tensor_tensor(out=smn, in0=smp[0:64, :], in1=i64,
                                        op=OP.mult)
                # ---- smalls pipeline ----
                sms = p_sm.tile([64, ts], F16, tag="sms")
                nc.vector.tensor_tensor_scan(
                    out=sms, data0=lam_col[0:64].broadcast_to([64, ts]),
                    data1=smn, initial=c_sm, op0=OP.mult, op1=OP.add)
                nc.gpsimd.tensor_copy(out=c_sm, in_=sms[:, ts - 1:ts])
                sigp = p_sm.tile([32, ts], F32, tag="sigp")
                nc.scalar.activation(out=sigp, in_=sms[0:32, :], func=AF.Tanh)
                om = p_sm.tile([32, ts], F32, tag="om")
                nc.gpsimd.tensor_scalar(out=om, in0=sigp, scalar1=0.02,
                                        scalar2=co["omega_base"],
                                        op0=OP.mult, op1=OP.add)
                nc.gpsimd.tensor_scalar(out=om, in0=om, scalar1=1.0,
                                        scalar2=0.001, op0=OP.min, op1=OP.max)
                phr = p_sm.tile([32, ts], F32, tag="phr")
                nc.vector.tensor_tensor_scan(
                    out=phr, data0=one_col[0:32].broadcast_to([32, ts]),
                    data1=om, initial=c_phi, op0=OP.mult, op1=OP.add)
                # wrap to [-pi, pi] via round-to-nearest int convert
                sc = p_sm.tile([64, ts], F16, tag="sc")
                for (half, ofs, bias) in ((0, 0.0, None), (1, 0.25, halfpi)):
                    wf = p_sm.tile([32, ts], F32, tag=f"wf{half}")
                    if ofs:
                        nc.gpsimd.tensor_scalar(
                            out=wf, in0=phr, scalar1=(1.0 / TWO_PI),
                            scalar2=ofs, op0=OP.mult, op1=OP.add)
                    else:
                        nc.gpsimd.tensor_scalar(
                            out=wf, in0=phr, scalar1=(1.0 / TWO_PI),
                            scalar2=None, op0=OP.mult)
                    wi = p_sm.tile([32, ts], I32, tag=f"wi{half}")
                    nc.vector.tensor_copy(out=wi, in_=wf)
                    nc.vector.tensor_copy(out=wf, in_=wi)
                    wrap = p_sm.tile([32, ts], F32, tag=f"wrap{half}")
                    nc.vector.scalar_tensor_tensor(
                        out=wrap, in0=wf, scalar=-TWO_PI, in1=phr,
                        op0=OP.mult, op1=OP.add)
                    if bias is None:
                        nc.scalar.activation(out=sc[0:32, :], in_=wrap,
                                             func=AF.Sin)
                    else:
                        nc.scalar.activation(out=sc[32:64, :], in_=wrap,
                                             func=AF.Sin, bias=bias)
                # carry: c_phi = wrapped phr last column
                cwf = p_sm.tile([32, 1], F32, tag="cwf")
                cwi = p_sm.tile([32, 1], I32, tag="cwi")
                nc.vector.tensor_scalar(out=cwf, in0=phr[:, ts - 1:ts],
                                        scalar1=(1.0 / TWO_PI),
                                        scalar2=None, op0=OP.mult)
                nc.vector.tensor_copy(out=cwi, in_=cwf)
                nc.vector.tensor_copy(out=cwf, in_=cwi)
                nc.vector.scalar_tensor_tensor(out=c_phi, in0=cwf, scalar=-TWO_PI,
                                               in1=phr[:, ts - 1:ts],
                                               op0=OP.mult, op1=OP.add)
                wh = p_sm.tile([32, ts], F16, tag="wh")
                nc.scalar.activation(out=wh, in_=sms[32:64, :], func=AF.Tanh,
                                     scale=0.5, bias=dgb_col[32:64])
                w0 = p_sm.tile([32, ts], F16, tag="w0")
                nc.gpsimd.tensor_scalar(out=w0, in0=wh, scalar1=0.5,
                                        scalar2=0.5, op0=OP.mult, op1=OP.add)

                # ---- per-superset feature pipeline ----
                for s in range(NSUP):
                    pe = [ps_pe.tile([128, ts], F32, tag="pe", name=f"pe{F}")
                          for F in range(2)]
                    for F in range(2):
                        for pw in range(4):
                            p = 4 * s + pw
                            nc.tensor.matmul(
                                out=pe[F],
                                lhsT=ct["c_we"][:, (F * 4 + pw) * 128:
                                                (F * 4 + pw + 1) * 128],
                                rhs=xTs[p], start=(pw == 0), stop=(pw == 3))
                        if co["has_proj_bias"]:
                            nc.tensor.matmul(
                                out=pe[F],
                                lhsT=ct["c_ebias"][:, 128 * F:128 * (F + 1)],
                                rhs=ones_row, start=False, stop=True,
                                skip_group_check=True)
                        nc.tensor.matmul(
                            out=pe[F],
                            lhsT=ct["c_mue"][:, 128 * (s * 2 + F):
                                             128 * (s * 2 + F + 1)],
                            rhs=mu_h, start=False, stop=True,
                            skip_group_check=True)
                    ibp = ps_sml.tile([128, ts], F32, tag="ib")
                    nc.tensor.matmul(out=ibp,
                                     lhsT=ct["c_binv"][:, 128 * s:128 * (s + 1)],
                                     rhs=inv_h, start=True, stop=True)
                    ib = p_st.tile([128, ts], F16, tag="ibs")
                    nc.scalar.copy(out=ib, in_=ibp)
                    peh = p_en.tile([128, 2, ts], F16, tag="peh")
                    for F in range(2):
                        nc.scalar.copy(out=peh[:, F, :], in_=pe[F])
                    en = p_en.tile([128, 2, ts], F16, tag="en")
                    nc.vector.tensor_tensor(
                        out=en, in0=peh,
                        in1=ib[:, None, :].broadcast_to([128, 2, ts]),
                        op=OP.mult)
                    ez = p_ez.tile([128, 2, ts], F16, tag="ez")
                    for F in range(2):
                        sk = 2 * s + F
                        nc.vector.tensor_tensor_scan(
                            out=ez[:, F, :],
                            data0=lam_col.broadcast_to([128, ts]),
                            data1=en[:, F, :], initial=c_ez[:, sk:sk + 1],
                            op0=OP.mult, op1=OP.add)
                        nc.gpsimd.tensor_copy(out=c_ez[:, sk:sk + 1],
                                              in_=ez[:, F, ts - 1:ts])
                    # phase features + e assembly
                    ep = [ps_pe.tile([128, ts], F32, tag="pe", name=f"ep{F}")
                          for F in range(2)]
                    eh = p_eh.tile([128, 2, ts + 1], F16, tag="eh")
                    nc.gpsimd.tensor_copy(out=eh[:, :, 0:1],
                                          in_=c_ep[:, 2 * s:2 * s + 2, None])
                    ephh = p_eh.tile([128, 2, ts], F16, tag="ephh")
                    for F in range(2):
                        nc.tensor.matmul(
                            out=ep[F],
                            lhsT=ct["c_eph"][:, 128 * (s * 2 + F):
                                             128 * (s * 2 + F + 1)],
                            rhs=sc, start=True, stop=True)
                        if co["has_berr"]:
                            nc.vector.scalar_tensor_tensor(
                                out=eh[:, F, 1:ts + 1], in0=ep[F],
                                scalar=ct["c_berr"][:, F:F + 1],
                                in1=ez[:, F, :], op0=OP.add, op1=OP.add)
                        else:
                            nc.scalar.copy(out=ephh[:, F, :], in_=ep[F])
                    if not co["has_berr"]:
                        nc.vector.tensor_tensor(
                            out=eh[:, :, 1:ts + 1], in0=ephh, in1=ez,
                            op=OP.add)
                    nc.gpsimd.tensor_copy(
                        out=c_ep[:, 2 * s:2 * s + 2, None],
                        in_=eh[:, :, ts:ts + 1])
                    si = p_si.tile([128, 2, ts], F16, tag="si")
                    for F in range(2):
                        sk = 2 * s + F
                        nc.vector.tensor_tensor_scan(
                            out=si[:, F, :],
                            data0=lam2_col.broadcast_to([128, ts]),
                            data1=eh[:, F, 1:ts + 1],
                            initial=c_si[:, sk:sk + 1],
                            op0=OP.mult, op1=OP.add)
                        nc.gpsimd.tensor_copy(out=c_si[:, sk:sk + 1],
                                              in_=si[:, F, ts - 1:ts])
                    t1s = p_yk.tile([128, 2, ts], F16, tag="t1s")
                    nc.gpsimd.tensor_scalar(out=t1s, in0=si,
                                            scalar1=co["ki_c1"], scalar2=None,
                                            op0=OP.mult)
                    yk = p_yk.tile([128, 2, ts], F16, tag="yk")
                    nc.vector.tensor_tensor(out=yk, in0=t1s,
                                            in1=eh[:, :, 1:ts + 1], op=OP.add)
                    t2s = p_yk.tile([128, 2, ts], F16, tag="t2s")
                    nc.gpsimd.tensor_scalar(out=t2s, in0=eh[:, :, 0:ts],
                                            scalar1=-co["kd_c1"], scalar2=None,
                                            op0=OP.mult)
                    nc.vector.tensor_tensor(out=yk, in0=yk, in1=t2s, op=OP.add)
                    # C = ykS + w0 * ykD
                    w0p = ps_sml.tile([128, ts], F32, tag="ib", name="w0p")
                    nc.tensor.matmul(
                        out=w0p, lhsT=ct["c_w0b"][:, 128 * s:128 * (s + 1)],
                        rhs=w0, start=True, stop=True)
                    tD = p_en.tile([128, ts], F16, tag="tD")
                    nc.vector.tensor_tensor(out=tD, in0=w0p, in1=yk[:, 0, :],
                                            op=OP.mult)
                    nc.vector.tensor_tensor(
                        out=ca[:, s, R + t0: R + t0 + ts], in0=tD,
                        in1=yk[:, 1, :], op=OP.add)

        # ================= overlap-save sweep =================
        with ExitStack() as phbc:
            paw = phbc.enter_context(tc.tile_pool(name="paw", bufs=1))
            a_wide = paw.tile([128, NSUP, ncol], F32)
            with ExitStack() as phb:
                swp = phb.enter_context(tc.tile_pool(name="swp", bufs=3))
                swp8 = phb.enter_context(tc.tile_pool(name="swp8", bufs=2))
                ca4 = ca.rearrange("p g (c r) -> p g c r", r=R)
                aw4 = a_wide.rearrange("p g (c r) -> p g c r", r=R)
                nh = nch // 2

                def tsl(t4, j, grp):
                    # column j in [R-W, 2R-W) of each chunk's window
                    if j < R:
                        return t4[:, :, grp:nch:2, j]
                    return t4[:, :, 1 + grp:nch + 1:2, j - R]

                for grp in range(2):
                    nc.vector.memset(tsl(aw4, R - W - 1, grp), 0.0)
                # D pre-scaled by kappa; updated once per DP-step block
                # (at the block's first step, effective from the next block;
                # validated: D lag up to 2*DP changes nothing) with
                # geometric-sum coefficients.  tanh(C - D) is batched: u8/h8
                # cover a whole DP block in one TT/Activation pair, computed
                # a block ahead so the per-step chain is only q -> rr -> a'.
                eff = sum(co["lam2"] ** jj for jj in range(DP))
                lam2dp = co["lam2"] ** DP
                nblk = (R + W) // DP
                assert nblk * DP == R + W and (R - W) % DP == 0

                def blk_ca(jb):
                    # merged-group slice [128, NSUP, nch, DP] at cols jb..jb+DP
                    if jb < R:
                        return ca4[:, :, 0:nch, jb:jb + DP]
                    return ca4[:, :, 1:nch + 1, jb - R:jb - R + DP]

                d_prev = [None, None]
                rr_prev = [None, None]
                p_next = {}          # (grp, j) -> precomputed h(j) - a(j-2)
                h8 = None
                h8n = None
                for blk in range(nblk):
                    jb = (R - W) + blk * DP
                    if blk == 0:
                        h8 = swp8.tile([128, NSUP, nch, DP], F32, tag="h8")
                        nc.scalar.activation(out=h8, in_=blk_ca(jb),
                                             func=AF.Tanh)
                    h8_cur, h8_nxt = h8, None
                    for k in range(DP):
                        j = jb + k
                        i = blk * DP + k

                        def hslice(kk, grp):
                            if kk < DP:
                                return h8_cur[:, :, grp::2, kk]
                            return h8_nxt[:, :, grp::2, kk - DP]

                        # loop-carried chain: rr(j-1) -> q(j) -> rr(j).
                        # q first in each engine queue.
                        qs = []
                        for grp in range(2):
                            q = swp.tile([128, NSUP, nh], F32, tag=f"q{grp}")
                            if i < 2:
                                nc.vector.tensor_tensor(
                                    out=q, in0=hslice(k, grp),
                                    in1=tsl(aw4, j - 1, grp), op=OP.subtract)
                            else:
                                nc.vector.scalar_tensor_tensor(
                                    out=q, in0=rr_prev[grp],
                                    scalar=-co["rate"],
                                    in1=p_next.pop((grp, j)),
                                    op0=OP.mult, op1=OP.add)
                            qs.append(q)
                        rrs = []
                        for grp in range(2):
                            rr = swp.tile([128, NSUP, nh], F32, tag=f"r{grp}")
                            nc.scalar.activation(out=rr, in_=qs[grp],
                                                 func=AF.Tanh, scale=co["s2"])
                            rrs.append(rr)
                            rr_prev[grp] = rr
                        for grp in range(2):
                            nc.vector.scalar_tensor_tensor(
                                out=tsl(aw4, j, grp), in0=rrs[grp],
                                scalar=co["rate"], in1=tsl(aw4, j - 1, grp),
                                op0=OP.mult, op1=OP.add)
                        # off-chain: p(j+2) = h(j+2) - a(j), on Pool
                        if i + 2 < R + W:
                            for grp in range(2):
                                p = swp.tile([128, NSUP, nh], F32,
                                             tag=f"p{grp}")
                                nc.gpsimd.tensor_tensor(
                                    out=p, in0=hslice(k + 2, grp),
                                    in1=tsl(aw4, j, grp), op=OP.subtract)
                                p_next[(grp, j + 2)] = p
                        if k == 0 and blk < nblk - 1:
                            # D update from this step's (q, rr); u8/h8 for the
                            # next block (split across Pool/DVE; 7 steps of
                            # slack to hide it)
                            for grp in range(2):
                                tq = swp.tile([128, NSUP, nh], F32,
                                              tag=f"tq{grp}")
                                nc.gpsimd.tensor_scalar(
                                    out=tq, in0=qs[grp],
                                    scalar1=-eff * co["kb"],
                                    scalar2=None, op0=OP.mult)
                                t1 = swp.tile([128, NSUP, nh], F32,
                                              tag=f"t1{grp}")
                                nc.vector.scalar_tensor_tensor(
                                    out=t1, in0=rrs[grp],
                                    scalar=eff * co["kr"],
                                    in1=tq, op0=OP.mult, op1=OP.add)
                                if d_prev[grp] is None:
                                    d_prev[grp] = t1
                                else:
                                    d_new = swp.tile([128, NSUP, nh], F32,
                                                     tag=f"dn{grp}")
                                    nc.vector.scalar_tensor_tensor(
                                        out=d_new, in0=d_prev[grp],
                                        scalar=lam2dp, in1=t1,
                                        op0=OP.mult, op1=OP.add)
                                    d_prev[grp] = d_new
                            njb = jb + DP
                            u8 = swp8.tile([128, NSUP, nch, DP], F32,
                                           tag="u8")
                            cblk = blk_ca(njb)
                            for grp in range(2):
                                eng = nc.gpsimd if grp == 0 else nc.vector
                                eng.tensor_tensor(
                                    out=u8[:, :, grp::2, :],
                                    in0=cblk[:, :, grp::2, :],
                                    in1=d_prev[grp][:, :, :, None]
                                    .broadcast_to([128, NSUP, nh, DP]),
                                    op=OP.subtract)
                            h8n = swp8.tile([128, NSUP, nch, DP], F32,
                                            tag="h8", name="h8n")
                            nc.scalar.activation(out=h8n, in_=u8,
                                                 func=AF.Tanh)
                            h8_nxt = h8n
                    h8 = h8_nxt if h8_nxt is not None else h8_cur

            # ============= output transpose + store =============
            with ExitStack() as phc:
                ps_o = phc.enter_context(tc.tile_pool(name="pso", bufs=2,
                                                      space="PSUM"))
                p_o = phc.enter_context(tc.tile_pool(name="po", bufs=3))
                for tau in range(t_total // 128):
                    tp = ps_o.tile([128, NSUP, 128], F32, tag="otp")
                    for g in range(NSUP):
                        nc.tensor.transpose(
                            tp[:, g, :],
                            a_wide[:, g, R + 128 * tau: R + 128 * (tau + 1)],
                            ident)
                    ot = p_o.tile([128, NSUP * 128], F32, tag="ot")
                    if tau % 2 == 0:
                        nc.scalar.copy(out=ot,
                                       in_=tp.rearrange("p g c -> p (g c)"))
                    else:
                        nc.vector.tensor_copy(
                            out=ot, in_=tp.rearrange("p g c -> p (g c)"))
                    nc.sync.dma_start(
                        out=out_d[128 * tau: 128 * (tau + 1)]
                        .rearrange("t b a -> t (b a)"), in_=ot)
    return nc


def _in_maps(inputs, consts):
    x = np.ascontiguousarray(np.asarray(inputs["states"], np.float32))
    maps = []
    for j in range(NCORES):
        m = {"x": np.ascontiguousarray(x[:, BL * j: BL * (j + 1), :])}
        m.update(consts)
        maps.append(m)
    return maps


def kernel(**inputs):
    co, consts = _coeffs(inputs)
    nc = bacc.Bacc("TRN2", num_devices=NCORES)
    build_program(nc, co)
    nc.compile()
    maps = _in_maps(inputs, consts)
    res = run_bass_kernel_spmd(nc, maps, list(range(NCORES)))
    outs = [np.asarray(res.results[j]["out"]).reshape(T_FULL, BL, A)
            for j in range(NCORES)]
    return np.concatenate(outs, axis=1)
